# revision 34
# baseline (speedup 1.0000x reference)
"""Trainium2 Bass kernel for nn_AtomicBasis_85263690760900.

Math: the reference reduces (after algebraic simplification and exploiting
that `channel_weights` feeds dead code only) to

    A_r[c, comp] = sum_i R[i,c] * (polynomial in u_i of degree <= 2) * h_blk[i,c,comp']

with R the (faithfully scrambled) Bessel radial basis and u the unit relative
positions.  Every output term is a "moment"  sum_i B_m[i,c] * h_blk[i,c,j]
for monomials B_m = R * {1, u_d, u_d u_e}.  On device we build the 10
monomial tensors (VectorE/ScalarE) and contract them against the raw
h-blocks with TensorE (per channel-group block-diagonal matmuls, PSUM
accumulation over neighbour tiles).  The host extracts the (c'==c) diagonal
entries and applies the small constant coefficient table.

Sharding: data-parallel over neighbours, 512 per core, 8 cores.  The radial
scramble R[i,c] = sqrt(2/5)*sin((i//64+1)*pi*d[64*(i%64)+c]/5)/d[64*(i%64)+c]
needs all 4096 distances on every core (rel_poss is only 48 KiB).

sin on ScalarE is only valid on [-pi, pi]; arguments reach ~200 rad, so we
range-reduce with the 2^23 float-magic round trick:
    t = d * (nu/10);  r = round(t) = (t + 2^23) - 2^23;  sin(x) = sin(-2pi*(r-t))
"""

import math

import numpy as np

import concourse.bass as bass
import concourse.tile as tile
from concourse import bacc, mybir
from concourse.bass_utils import run_bass_kernel_spmd

F32 = mybir.dt.float32
NCORES = 8
N, C, DIM = 4096, 64, 3
NSH = N // NCORES          # 512 neighbours per core
TILES, P = NSH // 128, 128  # 4 tiles of 128 partitions
MON = 10                    # R, R*u0..2, R*uu for 6 symmetric pairs
SYM = [(0, 0), (0, 1), (0, 2), (1, 1), (1, 2), (2, 2)]
GROUPS = [(0, 12), (12, 12), (24, 12), (36, 12), (48, 12), (60, 4)]
MAGIC = 12582912.0          # 1.5 * 2^23: float round-to-nearest trick
R_CUT = 5.0
N_WARMUP = 22               # dummy bf16 matmuls to release the PE clock gate

_UU_IDX = {}
for _m, (_a, _b) in enumerate(SYM):
    _UU_IDX[(_a, _b)] = 4 + _m
    _UU_IDX[(_b, _a)] = 4 + _m


def build_nc():
    nc = bacc.Bacc("TRN2", target_bir_lowering=False, debug=False,
                   num_devices=NCORES)

    hall = nc.dram_tensor("hall", [P, TILES * 832], F32, kind="ExternalInput")
    relp = nc.dram_tensor("relp", [P, 192], F32, kind="ExternalInput")
    relo = nc.dram_tensor("relo", [NSH, DIM], F32, kind="ExternalInput")
    nu10 = nc.dram_tensor("nu10", [P, TILES], F32, kind="ExternalInput")
    # group outputs packed side by side, two DMAs (groups 0-2 / 3-5)
    out_a = nc.dram_tensor("out_a", [MON * 12, 156 * 3], F32, kind="ExternalOutput")
    out_b = nc.dram_tensor("out_b", [MON * 12, 156 * 3], F32, kind="ExternalOutput")

    with tile.TileContext(nc) as tc:
        with (
            tc.tile_pool(name="big", bufs=1) as big,
            tc.tile_pool(name="small", bufs=1) as small,
            tc.tile_pool(name="obuf", bufs=1) as obuf,
            tc.tile_pool(name="psum", bufs=1, space="PSUM") as pp,
            tc.tile_pool(name="psumw", bufs=1, space="PSUM") as ppw,
        ):
            # ---- loads (small compute-critical tensors first; hall per tile
            # so matmuls for tile t only wait on their own slice) ----
            rp = small.tile([P, 192], F32)        # rel_poss grouped (q, c*3+d), dup halves
            nc.sync.dma_start(out=rp[:], in_=relp[:])

            rpo = small.tile([P, TILES, DIM], F32)  # own rel_poss, (p, t, d)
            nc.sync.dma_start(out=rpo[:], in_=relo[:].rearrange("(t p) d -> p t d", p=P))

            nt = small.tile([P, TILES], F32)
            nc.sync.dma_start(out=nt[:], in_=nu10[:])

            HH = big.tile([P, TILES, 832], F32)   # [h0|h1|h2] channel-major
            hall_v = hall[:].rearrange("p (t m) -> p t m", t=TILES)
            for t in range(TILES):
                nc.sync.dma_start(out=HH[:, t, :], in_=hall_v[:, t, :])

            # ---- distances for the scrambled radial basis (DVE/ACT) ----
            sq = small.tile([P, 192], F32)
            nc.vector.tensor_mul(sq[:], rp[:], rp[:])
            ssq = small.tile([P, C], F32)
            nc.vector.reduce_sum(ssq[:], sq[:].rearrange("p (c d) -> p c d", d=DIM),
                                 axis=mybir.AxisListType.X)
            dd = small.tile([P, C], F32)          # dd[p, c] = d[64*(p%64)+c]
            nc.scalar.sqrt(dd[:], ssq[:])
            inv = small.tile([P, C], F32)
            nc.vector.reciprocal(inv[:], dd[:])

            # ---- own-neighbour unit vectors & uu pairs ----
            sqo = small.tile([P, TILES, DIM], F32)
            nc.vector.tensor_mul(sqo[:], rpo[:], rpo[:])
            sso = small.tile([P, TILES], F32)
            nc.vector.reduce_sum(sso[:], sqo[:], axis=mybir.AxisListType.X)
            do = small.tile([P, TILES], F32)
            nc.scalar.sqrt(do[:], sso[:])
            invo = small.tile([P, TILES], F32)
            nc.vector.reciprocal(invo[:], do[:])
            u = small.tile([P, TILES, DIM], F32)
            nc.vector.tensor_mul(
                u[:], rpo[:], invo[:, :, None].broadcast_to([P, TILES, DIM]))
            uu6 = small.tile([P, TILES, 6], F32)  # sym pairs 00,01,02,11,12,22
            nc.vector.tensor_mul(
                uu6[:, :, 0:3],
                u[:, :, 0:1].broadcast_to([P, TILES, 3]), u[:, :, 0:3])
            nc.vector.tensor_mul(
                uu6[:, :, 3:5],
                u[:, :, 1:2].broadcast_to([P, TILES, 2]), u[:, :, 1:3])
            nc.vector.tensor_mul(uu6[:, :, 5:6], u[:, :, 2:3], u[:, :, 2:3])

            # ---- PE warm-up: the HAM clock gate keeps TensorE at half rate
            # until ~4us of sustained activity; burn dummy bf16 matmuls in
            # the otherwise idle DMA/DVE window so real matmuls run at 2.4GHz
            BF16 = mybir.dt.bfloat16
            ww = small.tile([P, P], BF16)
            wr = small.tile([P, 512], BF16)
            nc.vector.memset(ww[:], 0.0)
            nc.vector.memset(wr[:], 0.0)
            wp = ppw.tile([P, 512], F32, tag="warm")
            for _ in range(N_WARMUP):
                nc.tensor.matmul(wp[:], ww[:], wr[:], start=True, stop=True)

            # ---- monomials (stationary side), built per tile-PAIR so the
            # first matmuls can start while the second half still builds.
            # channel-major (c, m) so the weight slice is one free dim.
            LHS = big.tile([P, TILES, C, MON], F32)
            t_ = small.tile([P, TILES, C], F32)
            tb = small.tile([P, TILES, C], F32)
            fr = small.tile([P, TILES, C], F32)   # fr = round(t) - t in [-.5, .5]
            sinb = small.tile([P, TILES, C], F32)
            HALves = [(0, 2), (2, 2)]
            for (h0_, nh) in HALves:
                ts_ = slice(h0_, h0_ + nh)
                nc.vector.tensor_mul(
                    t_[:, ts_, :],
                    dd[:, None, :].broadcast_to([P, nh, C]),
                    nt[:, ts_, None].broadcast_to([P, nh, C]))
                nc.vector.tensor_scalar_add(tb[:, ts_, :], t_[:, ts_, :], MAGIC)
                nc.vector.scalar_tensor_tensor(
                    fr[:, ts_, :], tb[:, ts_, :], MAGIC, t_[:, ts_, :],
                    op0=mybir.AluOpType.subtract, op1=mybir.AluOpType.subtract)
                nc.scalar.activation(sinb[:, ts_, :], fr[:, ts_, :],
                                     mybir.ActivationFunctionType.Sin,
                                     scale=-2.0 * math.pi)
                nc.vector.tensor_mul(
                    LHS[:, ts_, :, 0], sinb[:, ts_, :],
                    inv[:, None, :].broadcast_to([P, nh, C]))
                nc.vector.tensor_mul(
                    LHS[:, ts_, :, 1:4],
                    LHS[:, ts_, :, 0:1].broadcast_to([P, nh, C, 3]),
                    u[:, ts_, None, :].broadcast_to([P, nh, C, 3]))
                nc.vector.tensor_mul(
                    LHS[:, ts_, :, 4:7],
                    LHS[:, ts_, :, 0:1].broadcast_to([P, nh, C, 3]),
                    uu6[:, ts_, None, 0:3].broadcast_to([P, nh, C, 3]))
                nc.gpsimd.tensor_mul(
                    LHS[:, ts_, :, 7:10],
                    LHS[:, ts_, :, 0:1].broadcast_to([P, nh, C, 3]),
                    uu6[:, ts_, None, 3:6].broadcast_to([P, nh, C, 3]))

            # ---- block-diagonal moment matmuls.  Tiles 0-1 first for all
            # groups (overlapping the half-2 LHS build), then per-group
            # tiles 2-3 with each group's output streamed immediately ----
            pss = []
            for gi, (c0, G) in enumerate(GROUPS):
                ps_g = pp.tile([MON * G, 13 * G], F32, tag=f"ps{gi}")
                pss.append(ps_g)
            for t in range(2):
                for gi, (c0, G) in enumerate(GROUPS):
                    nc.tensor.matmul(pss[gi][:], LHS[:, t, c0:c0 + G, :],
                                     HH[:, t, 13 * c0: 13 * (c0 + G)],
                                     start=(t == 0), stop=False)
            oba = obuf.tile([MON * 12, 156 * 3], F32, tag="oba")
            obb = obuf.tile([MON * 12, 156 * 3], F32, tag="obb")
            for gi, (c0, G) in enumerate(GROUPS):
                for t in range(2, TILES):
                    nc.tensor.matmul(pss[gi][:], LHS[:, t, c0:c0 + G, :],
                                     HH[:, t, 13 * c0: 13 * (c0 + G)],
                                     start=False, stop=(t == TILES - 1))
                ob, col = (oba, gi) if gi < 3 else (obb, gi - 3)
                nc.scalar.copy(ob[0:MON * G, 156 * col: 156 * col + 13 * G],
                               pss[gi][:])
                if gi == 2:
                    nc.sync.dma_start(out=out_a[:], in_=oba[:])
            nc.scalar.dma_start(out=out_b[:], in_=obb[:])

    nc.compile()
    return nc


_NC_CACHE = None


def _get_nc():
    global _NC_CACHE
    if _NC_CACHE is None:
        _NC_CACHE = build_nc()
    return _NC_CACHE


def make_in_maps(h0, h1, h2, rel_poss):
    f32 = np.float32
    h0 = np.asarray(h0, f32).reshape(N, C)
    h1 = np.asarray(h1, f32).reshape(N, C, DIM)
    h2 = np.asarray(h2, f32).reshape(N, C, DIM * DIM)
    rel = np.asarray(rel_poss, f32).reshape(N, DIM)

    # channel-major pack: hall[i, c*13 + j], j: [h0 | h1(3) | h2(9)]
    hall = np.empty((N, C, 13), f32)
    hall[:, :, 0] = h0
    hall[:, :, 1:4] = h1
    hall[:, :, 4:13] = h2
    hall = hall.reshape(N, 832)

    relp = np.ascontiguousarray(
        np.concatenate([rel.reshape(64, 192)] * 2, axis=0))  # dup partition halves

    in_maps = []
    for k in range(NCORES):
        sl = slice(NSH * k, NSH * (k + 1))
        hck = np.ascontiguousarray(
            hall[sl].reshape(TILES, P, 832).transpose(1, 0, 2).reshape(P, TILES * 832))
        nu = np.empty((P, TILES), f32)
        for t in range(TILES):
            nu[:64, t] = (8 * k + 2 * t + 1) / 10.0
            nu[64:, t] = (8 * k + 2 * t + 2) / 10.0
        in_maps.append({
            "hall": hck,
            "relp": relp,
            "relo": np.ascontiguousarray(rel[sl]),
            "nu10": nu,
        })
    return in_maps


def combine(results):
    """results: list (per core) of dicts out0..out5 -> (A0, A1, A2)."""
    S0 = np.zeros((MON, C), np.float64)
    S1 = np.zeros((MON, C, 3), np.float64)
    S2 = np.zeros((MON, C, 9), np.float64)
    for res in results:
        oa = np.concatenate([np.asarray(res["out_a"], np.float64),
                             np.asarray(res["out_b"], np.float64)], axis=1)
        for gi, (c0, G) in enumerate(GROUPS):
            o = oa[0:MON * G, 156 * gi: 156 * gi + 13 * G].reshape(G, MON, G, 13)
            diag = o[np.arange(G), :, np.arange(G), :]       # (G, MON, 13)
            S0[:, c0:c0 + G] += diag[:, :, 0].T
            S1[:, c0:c0 + G, :] += diag[:, :, 1:4].transpose(1, 0, 2)
            S2[:, c0:c0 + G, :] += diag[:, :, 4:13].transpose(1, 0, 2)

    A0 = np.zeros(C, np.float64)
    A1 = np.zeros((C, 3), np.float64)
    A2 = np.zeros((C, 3, 3), np.float64)

    A0 += 2 * S0[0]
    for d in range(3):
        A0 += S1[1 + d, :, d]
        A0 += 2 * S2[0, :, 3 * d + d]
        for e in range(3):
            A0 += 2 * S2[_UU_IDX[(d, e)], :, 3 * d + e]

    for d in range(3):
        A1[:, d] += S0[1 + d]
        A1[:, d] += 2 * S1[0, :, d]
        for e in range(3):
            A1[:, d] += 2 * S1[_UU_IDX[(d, e)], :, e]
            A1[:, d] += S2[1 + d, :, 3 * e + e]
            A1[:, d] += S2[1 + e, :, 3 * d + e] + S2[1 + e, :, 3 * e + d]

    for d in range(3):
        for e in range(3):
            A2[:, d, e] += S0[_UU_IDX[(d, e)]]
            A2[:, d, e] += S1[1 + e, :, d]
            A2[:, d, e] += 2 * S2[0, :, 3 * d + e]
            for f in range(3):
                A2[:, d, e] += S2[_UU_IDX[(d, e)], :, 3 * f + f]
                A2[:, d, e] += 2 * (S2[_UU_IDX[(f, e)], :, 3 * d + f] +
                                    S2[_UU_IDX[(f, e)], :, 3 * f + d])

    s = math.sqrt(2.0 / R_CUT)
    return ((s * A0).astype(np.float32),
            (s * A1).astype(np.float32),
            (s * A2).astype(np.float32))


def run_device(in_maps, **kwargs):
    nc = _get_nc()
    return run_bass_kernel_spmd(nc, in_maps, core_ids=list(range(NCORES)), **kwargs)


def kernel(h0, h1, h2, rel_poss, channel_weights=None):
    in_maps = make_in_maps(h0, h1, h2, rel_poss)
    res = run_device(in_maps)
    return combine(res.results)


# revision 38
# speedup vs baseline: 1.0520x; 1.0520x over previous
"""Trainium2 Bass kernel for nn_AtomicBasis_85263690760900.

Math: the reference reduces (after algebraic simplification and exploiting
that `channel_weights` feeds dead code only) to

    A_r[c, comp] = sum_i R[i,c] * (polynomial in u_i of degree <= 2) * h_blk[i,c,comp']

with R the (faithfully scrambled) Bessel radial basis and u the unit relative
positions.  Every output term is a "moment"  sum_i B_m[i,c] * h_blk[i,c,j]
for monomials B_m = R * {1, u_d, u_d u_e}.  On device we build the 10
monomial tensors (VectorE/ScalarE) and contract them against the raw
h-blocks with TensorE (per channel-group block-diagonal matmuls, PSUM
accumulation over neighbour tiles).  The host extracts the (c'==c) diagonal
entries and applies the small constant coefficient table.

Sharding: data-parallel over neighbours, 512 per core, 8 cores.  The radial
scramble R[i,c] = sqrt(2/5)*sin((i//64+1)*pi*d[64*(i%64)+c]/5)/d[64*(i%64)+c]
needs all 4096 distances on every core (rel_poss is only 48 KiB).

sin on ScalarE is only valid on [-pi, pi]; arguments reach ~200 rad, so we
range-reduce with the 2^23 float-magic round trick:
    t = d * (nu/10);  r = round(t) = (t + 2^23) - 2^23;  sin(x) = sin(-2pi*(r-t))
"""

import math

import numpy as np

import concourse.bass as bass
import concourse.tile as tile
from concourse import bacc, mybir
from concourse.bass_utils import run_bass_kernel_spmd

F32 = mybir.dt.float32
NCORES = 8
N, C, DIM = 4096, 64, 3
NSH = N // NCORES          # 512 neighbours per core
TILES, P = NSH // 128, 128  # 4 tiles of 128 partitions
MON = 10                    # R, R*u0..2, R*uu for 6 symmetric pairs
SYM = [(0, 0), (0, 1), (0, 2), (1, 1), (1, 2), (2, 2)]
GROUPS = [(0, 12), (12, 12), (24, 12), (36, 12), (48, 12), (60, 4)]
MAGIC = 12582912.0          # 1.5 * 2^23: float round-to-nearest trick
R_CUT = 5.0
N_WARMUP = 22               # dummy bf16 matmuls to release the PE clock gate

_UU_IDX = {}
for _m, (_a, _b) in enumerate(SYM):
    _UU_IDX[(_a, _b)] = 4 + _m
    _UU_IDX[(_b, _a)] = 4 + _m


def build_nc():
    nc = bacc.Bacc("TRN2", target_bir_lowering=False, debug=False,
                   num_devices=NCORES)

    hall = nc.dram_tensor("hall", [P, TILES * 832], F32, kind="ExternalInput")
    relp = nc.dram_tensor("relp", [P, 192], F32, kind="ExternalInput")
    relo = nc.dram_tensor("relo", [NSH, DIM], F32, kind="ExternalInput")
    nu10 = nc.dram_tensor("nu10", [P, TILES], F32, kind="ExternalInput")
    # group outputs packed pairwise -> three streamed output DMAs
    out_ts = [nc.dram_tensor(f"out{j}", [MON * 12, 156 * 2], F32,
                             kind="ExternalOutput") for j in range(3)]

    with tile.TileContext(nc) as tc:
        with (
            tc.tile_pool(name="big", bufs=1) as big,
            tc.tile_pool(name="small", bufs=1) as small,
            tc.tile_pool(name="obuf", bufs=1) as obuf,
            tc.tile_pool(name="psum", bufs=1, space="PSUM") as pp,
            tc.tile_pool(name="psumw", bufs=1, space="PSUM") as ppw,
        ):
            # ---- loads (small compute-critical tensors first; hall per tile
            # so matmuls for tile t only wait on their own slice) ----
            rp = small.tile([P, 192], F32)        # rel_poss grouped (q, c*3+d), dup halves
            nc.sync.dma_start(out=rp[:], in_=relp[:])

            rpo = small.tile([P, TILES, DIM], F32)  # own rel_poss, (p, t, d)
            nc.sync.dma_start(out=rpo[:], in_=relo[:].rearrange("(t p) d -> p t d", p=P))

            nt = small.tile([P, TILES], F32)
            nc.sync.dma_start(out=nt[:], in_=nu10[:])

            HH = big.tile([P, TILES, 832], F32)   # [h0|h1|h2] channel-major
            hall_v = hall[:].rearrange("p (t m) -> p t m", t=TILES)
            for t in range(TILES):
                nc.sync.dma_start(out=HH[:, t, :], in_=hall_v[:, t, :])

            # ---- distances for the scrambled radial basis (DVE/ACT) ----
            sq = small.tile([P, 192], F32)
            nc.vector.tensor_mul(sq[:], rp[:], rp[:])
            ssq = small.tile([P, C], F32)
            nc.vector.reduce_sum(ssq[:], sq[:].rearrange("p (c d) -> p c d", d=DIM),
                                 axis=mybir.AxisListType.X)
            dd = small.tile([P, C], F32)          # dd[p, c] = d[64*(p%64)+c]
            nc.scalar.sqrt(dd[:], ssq[:])
            inv = small.tile([P, C], F32)
            nc.vector.reciprocal(inv[:], dd[:])

            # ---- own-neighbour unit vectors & uu pairs ----
            sqo = small.tile([P, TILES, DIM], F32)
            nc.vector.tensor_mul(sqo[:], rpo[:], rpo[:])
            sso = small.tile([P, TILES], F32)
            nc.vector.reduce_sum(sso[:], sqo[:], axis=mybir.AxisListType.X)
            do = small.tile([P, TILES], F32)
            nc.scalar.sqrt(do[:], sso[:])
            invo = small.tile([P, TILES], F32)
            nc.vector.reciprocal(invo[:], do[:])
            u = small.tile([P, TILES, DIM], F32)
            nc.vector.tensor_mul(
                u[:], rpo[:], invo[:, :, None].broadcast_to([P, TILES, DIM]))
            uu6 = small.tile([P, TILES, 6], F32)  # sym pairs 00,01,02,11,12,22
            nc.vector.tensor_mul(
                uu6[:, :, 0:3],
                u[:, :, 0:1].broadcast_to([P, TILES, 3]), u[:, :, 0:3])
            nc.vector.tensor_mul(
                uu6[:, :, 3:5],
                u[:, :, 1:2].broadcast_to([P, TILES, 2]), u[:, :, 1:3])
            nc.vector.tensor_mul(uu6[:, :, 5:6], u[:, :, 2:3], u[:, :, 2:3])

            # ---- PE warm-up: the HAM clock gate keeps TensorE at half rate
            # until ~4us of sustained activity; burn dummy bf16 matmuls in
            # the otherwise idle DMA/DVE window so real matmuls run at 2.4GHz
            BF16 = mybir.dt.bfloat16
            ww = small.tile([P, P], BF16)
            wr = small.tile([P, 512], BF16)
            nc.vector.memset(ww[:], 0.0)
            nc.vector.memset(wr[:], 0.0)
            wp = ppw.tile([P, 512], F32, tag="warm")
            for _ in range(N_WARMUP):
                nc.tensor.matmul(wp[:], ww[:], wr[:], start=True, stop=True)

            # ---- monomials (stationary side), built per tile-PAIR so the
            # first matmuls can start while the second half still builds.
            # channel-major (c, m) so the weight slice is one free dim.
            LHS = big.tile([P, TILES, C, MON], F32)
            t_ = small.tile([P, TILES, C], F32)
            tb = small.tile([P, TILES, C], F32)
            fr = small.tile([P, TILES, C], F32)   # fr = round(t) - t in [-.5, .5]
            sinb = small.tile([P, TILES, C], F32)
            HALves = [(0, 2), (2, 2)]
            for (h0_, nh) in HALves:
                ts_ = slice(h0_, h0_ + nh)
                nc.vector.tensor_mul(
                    t_[:, ts_, :],
                    dd[:, None, :].broadcast_to([P, nh, C]),
                    nt[:, ts_, None].broadcast_to([P, nh, C]))
                nc.vector.tensor_scalar_add(tb[:, ts_, :], t_[:, ts_, :], MAGIC)
                nc.vector.scalar_tensor_tensor(
                    fr[:, ts_, :], tb[:, ts_, :], MAGIC, t_[:, ts_, :],
                    op0=mybir.AluOpType.subtract, op1=mybir.AluOpType.subtract)
                nc.scalar.activation(sinb[:, ts_, :], fr[:, ts_, :],
                                     mybir.ActivationFunctionType.Sin,
                                     scale=-2.0 * math.pi)
                nc.vector.tensor_mul(
                    LHS[:, ts_, :, 0], sinb[:, ts_, :],
                    inv[:, None, :].broadcast_to([P, nh, C]))
                nc.vector.tensor_mul(
                    LHS[:, ts_, :, 1:4],
                    LHS[:, ts_, :, 0:1].broadcast_to([P, nh, C, 3]),
                    u[:, ts_, None, :].broadcast_to([P, nh, C, 3]))
                nc.vector.tensor_mul(
                    LHS[:, ts_, :, 4:10],
                    LHS[:, ts_, :, 0:1].broadcast_to([P, nh, C, 6]),
                    uu6[:, ts_, None, :].broadcast_to([P, nh, C, 6]))

            # ---- block-diagonal moment matmuls.  Tiles 0-1 first for all
            # groups (overlapping the half-2 LHS build), then per-group
            # tiles 2-3 with each group's output streamed immediately ----
            pss = []
            for gi, (c0, G) in enumerate(GROUPS):
                ps_g = pp.tile([MON * G, 13 * G], F32, tag=f"ps{gi}")
                pss.append(ps_g)
            for t in range(2):
                for gi, (c0, G) in enumerate(GROUPS):
                    nc.tensor.matmul(pss[gi][:], LHS[:, t, c0:c0 + G, :],
                                     HH[:, t, 13 * c0: 13 * (c0 + G)],
                                     start=(t == 0), stop=False)
            obs = []
            for j in range(3):
                ob_j = obuf.tile([MON * 12, 156 * 2], F32, tag=f"ob{j}")
                obs.append(ob_j)
            for gi, (c0, G) in enumerate(GROUPS):
                for t in range(2, TILES):
                    nc.tensor.matmul(pss[gi][:], LHS[:, t, c0:c0 + G, :],
                                     HH[:, t, 13 * c0: 13 * (c0 + G)],
                                     start=False, stop=(t == TILES - 1))
                ob, col = obs[gi // 2], gi % 2
                nc.scalar.copy(ob[0:MON * G, 156 * col: 156 * col + 13 * G],
                               pss[gi][:])
                if col == 1:
                    nc.sync.dma_start(out=out_ts[gi // 2][:], in_=ob[:])

    nc.compile()
    return nc


_NC_CACHE = None


def _get_nc():
    global _NC_CACHE
    if _NC_CACHE is None:
        _NC_CACHE = build_nc()
    return _NC_CACHE


def make_in_maps(h0, h1, h2, rel_poss):
    f32 = np.float32
    h0 = np.asarray(h0, f32).reshape(N, C)
    h1 = np.asarray(h1, f32).reshape(N, C, DIM)
    h2 = np.asarray(h2, f32).reshape(N, C, DIM * DIM)
    rel = np.asarray(rel_poss, f32).reshape(N, DIM)

    # channel-major pack: hall[i, c*13 + j], j: [h0 | h1(3) | h2(9)]
    hall = np.empty((N, C, 13), f32)
    hall[:, :, 0] = h0
    hall[:, :, 1:4] = h1
    hall[:, :, 4:13] = h2
    hall = hall.reshape(N, 832)

    relp = np.ascontiguousarray(
        np.concatenate([rel.reshape(64, 192)] * 2, axis=0))  # dup partition halves

    in_maps = []
    for k in range(NCORES):
        sl = slice(NSH * k, NSH * (k + 1))
        hck = np.ascontiguousarray(
            hall[sl].reshape(TILES, P, 832).transpose(1, 0, 2).reshape(P, TILES * 832))
        nu = np.empty((P, TILES), f32)
        for t in range(TILES):
            nu[:64, t] = (8 * k + 2 * t + 1) / 10.0
            nu[64:, t] = (8 * k + 2 * t + 2) / 10.0
        in_maps.append({
            "hall": hck,
            "relp": relp,
            "relo": np.ascontiguousarray(rel[sl]),
            "nu10": nu,
        })
    return in_maps


def combine(results):
    """results: list (per core) of dicts out0..out5 -> (A0, A1, A2)."""
    S0 = np.zeros((MON, C), np.float64)
    S1 = np.zeros((MON, C, 3), np.float64)
    S2 = np.zeros((MON, C, 9), np.float64)
    for res in results:
        oa = np.concatenate([np.asarray(res[f"out{j}"], np.float64)
                             for j in range(3)], axis=1)
        for gi, (c0, G) in enumerate(GROUPS):
            o = oa[0:MON * G, 156 * gi: 156 * gi + 13 * G].reshape(G, MON, G, 13)
            diag = o[np.arange(G), :, np.arange(G), :]       # (G, MON, 13)
            S0[:, c0:c0 + G] += diag[:, :, 0].T
            S1[:, c0:c0 + G, :] += diag[:, :, 1:4].transpose(1, 0, 2)
            S2[:, c0:c0 + G, :] += diag[:, :, 4:13].transpose(1, 0, 2)

    A0 = np.zeros(C, np.float64)
    A1 = np.zeros((C, 3), np.float64)
    A2 = np.zeros((C, 3, 3), np.float64)

    A0 += 2 * S0[0]
    for d in range(3):
        A0 += S1[1 + d, :, d]
        A0 += 2 * S2[0, :, 3 * d + d]
        for e in range(3):
            A0 += 2 * S2[_UU_IDX[(d, e)], :, 3 * d + e]

    for d in range(3):
        A1[:, d] += S0[1 + d]
        A1[:, d] += 2 * S1[0, :, d]
        for e in range(3):
            A1[:, d] += 2 * S1[_UU_IDX[(d, e)], :, e]
            A1[:, d] += S2[1 + d, :, 3 * e + e]
            A1[:, d] += S2[1 + e, :, 3 * d + e] + S2[1 + e, :, 3 * e + d]

    for d in range(3):
        for e in range(3):
            A2[:, d, e] += S0[_UU_IDX[(d, e)]]
            A2[:, d, e] += S1[1 + e, :, d]
            A2[:, d, e] += 2 * S2[0, :, 3 * d + e]
            for f in range(3):
                A2[:, d, e] += S2[_UU_IDX[(d, e)], :, 3 * f + f]
                A2[:, d, e] += 2 * (S2[_UU_IDX[(f, e)], :, 3 * d + f] +
                                    S2[_UU_IDX[(f, e)], :, 3 * f + d])

    s = math.sqrt(2.0 / R_CUT)
    return ((s * A0).astype(np.float32),
            (s * A1).astype(np.float32),
            (s * A2).astype(np.float32))


def run_device(in_maps, **kwargs):
    nc = _get_nc()
    return run_bass_kernel_spmd(nc, in_maps, core_ids=list(range(NCORES)), **kwargs)


def kernel(h0, h1, h2, rel_poss, channel_weights=None):
    in_maps = make_in_maps(h0, h1, h2, rel_poss)
    res = run_device(in_maps)
    return combine(res.results)
